# revision 88
# baseline (speedup 1.0000x reference)
"""Trainium2 Bass kernel for Bahdanau-style attention scoring (sparse_attention).

Math (per reference):
    u1 = W[:, :H].T @ v ; u2 = W[:, H:].T @ v ; c = b @ v   (host, tiny)
    sh[b, n] = hidden[n, b, :] @ u1
    se[b, t] = encoder_outputs[t, b, :] @ u2
    out[b, n, t] = softmax_t(tanh(sh[b, n] + se[b, t] + c))

Sharding: data-parallel over batch B=64 across 8 cores (8 batch rows per
core); u1/u2/c replicated. No collectives.

v2 design (from v1 trace: DVE 53.8us / DMA 53us / ACT 52.3us all ~60%
busy, span 84us):
  - fp16 everywhere on chip (DVE 2x perf mode needs all-2B operands);
    fp16 output store halves store traffic: DMA ~53 -> ~42us.
  - enc/hid loaded fp32->fp16 via SWDGE cast-DMA (gpsimd), all up front.
  - DVE: dots at 2x (mul + grouped reduce, fp16 out under
    allow_low_precision), seT copy, recip, normalize j=0.
  - ACT: tanh (bias=sh+c per j) + exp with accum_out row sums.
  - GpSimd: normalize j=1 (tensor_scalar_mul), SWDGE descgen.
  - PE: u broadcast, se transpose, one-hot broadcast matmuls (fp16 1-pass).
"""

import os
import sys

import numpy as np

for _p in ("/opt/trn_rl_repo", "/root/.axon_site/_ro/trn_rl_repo"):
    if os.path.isdir(_p) and _p not in sys.path:
        sys.path.insert(0, _p)

from contextlib import ExitStack

import concourse.bass as bass
import concourse.tile as tile
from concourse import bacc, mybir
from concourse import masks
from concourse.bass_utils import run_bass_kernel_spmd

H = 256
N_LEN = 256
T_LEN = 1024
BATCH = 64
NCORES = 8
B_LOC = BATCH // NCORES  # 8
P = 128
FP32 = mybir.dt.float32
FP16 = mybir.dt.float16
AF = mybir.ActivationFunctionType
ALU = mybir.AluOpType
AX = mybir.AxisListType


def build_program():
    nc = bacc.Bacc(
        "TRN2",
        target_bir_lowering=False,
        debug=False,
        enable_asserts=True,
        num_devices=NCORES,
    )

    hid_ap = nc.dram_tensor("hidden", [N_LEN, B_LOC, H], FP32, kind="ExternalInput").ap()
    enc_ap = nc.dram_tensor(
        "encoder_outputs", [T_LEN, B_LOC, H], FP32, kind="ExternalInput"
    ).ap()
    ucat_ap = nc.dram_tensor("u_cat", [1, 2 * H], FP16, kind="ExternalInput").ap()
    c_ap = nc.dram_tensor("c_in", [1, 1], FP16, kind="ExternalInput").ap()
    sel_ap = nc.dram_tensor("sel_in", [8, 8 * P], FP16, kind="ExternalInput").ap()
    sel4_ap = nc.dram_tensor("sel4_in", [4, 8 * P], FP16, kind="ExternalInput").ap()
    ident_ap = nc.dram_tensor("ident_in", [P, P], FP16, kind="ExternalInput").ap()
    out_ap = nc.dram_tensor(
        "out", [B_LOC, N_LEN, T_LEN], FP16, kind="ExternalOutput"
    ).ap()

    # DRAM views: partition index innermost of the row dims.
    hid_r = hid_ap.rearrange("(j p) b h -> p b j h", p=P)  # (128, 8, 2, 256)
    enc_r = enc_ap.rearrange("(k p) b h -> p b k h", p=P)  # (128, 8, 8, 256)
    out_r = out_ap.rearrange("b (j p) t -> b p j t", p=P)  # (8, 128, 2, 1024)

    with tile.TileContext(nc) as tc, ExitStack() as ctx:
        singles = ctx.enter_context(tc.tile_pool(name="singles", bufs=1))
        ps_pre = ctx.enter_context(tc.tile_pool(name="ps_pre", bufs=3, space="PSUM"))
        ps_tr = ctx.enter_context(tc.tile_pool(name="ps_tr", bufs=2, space="PSUM"))
        enc_pool = ctx.enter_context(tc.tile_pool(name="enc", bufs=8))
        stats = ctx.enter_context(tc.tile_pool(name="stats", bufs=6))
        scratch_pool = ctx.enter_context(tc.tile_pool(name="scratch", bufs=2))
        et_pool = ctx.enter_context(tc.tile_pool(name="et", bufs=2))
        xt_pool = ctx.enter_context(tc.tile_pool(name="xt", bufs=3))
        ot_pool = ctx.enter_context(tc.tile_pool(name="ot", bufs=3))

        # ---- tiny loads first (HWDGE): u_cat row + c scalar ----
        ucat_sb = singles.tile([1, 2 * H], FP16)
        nc.sync.dma_start(ucat_sb[:], ucat_ap)
        c_sb = singles.tile([1, 1], FP16)
        nc.sync.dma_start(c_sb[:], c_ap)
        sel = singles.tile([8, 8, P], FP16)
        nc.sync.dma_start(sel[:].rearrange("a k p -> a (k p)"), sel_ap)
        sel4 = singles.tile([4, 8, P], FP16)
        nc.sync.dma_start(sel4[:].rearrange("a k p -> a (k p)"), sel4_ap)
        ident = singles.tile([P, P], FP16)
        nc.sync.dma_start(ident[:], ident_ap)

        # ---- big input loads, all up front (SWDGE cast fp32->fp16).
        # SWDGE DMAs drain in issue order and each has ~1us descgen plus
        # ~2us completion latency, so batch 0's enc is split in quarters
        # (first dot work can start ~11us in) and batch 1's in halves;
        # hid slices interleave so hid_b lands just before tanh_b needs
        # its bias ----
        hid_all = singles.tile([P, B_LOC, 2, H], FP16)
        enc_sbs = []
        for b in range(B_LOC):
            enc_sb = enc_pool.tile([P, 8, H], FP16)
            enc_sbs.append(enc_sb)
        nc.gpsimd.dma_start(hid_all[:, 0], hid_r[:, 0])
        # decreasing chunk sizes: the LAST chunk gates the first tanh, so
        # keep it tiny (lands earlier, and its dots are short)
        ENC0_SPLITS = ((0, 3), (3, 6), (6, 7), (7, 8))
        for lo, hi in ENC0_SPLITS:
            nc.gpsimd.dma_start(enc_sbs[0][:, lo:hi], enc_r[:, 0, lo:hi])
        for hh in range(2):
            nc.gpsimd.dma_start(
                enc_sbs[1][:, 4 * hh : 4 * hh + 4], enc_r[:, 1, 4 * hh : 4 * hh + 4]
            )
        hid_next = 1
        for b in range(2, B_LOC):
            for _ in range(2):
                if hid_next < B_LOC:
                    nc.gpsimd.dma_start(hid_all[:, hid_next], hid_r[:, hid_next])
                    hid_next += 1
            nc.gpsimd.dma_start(enc_sbs[b][:], enc_r[:, b])
        while hid_next < B_LOC:
            nc.gpsimd.dma_start(hid_all[:, hid_next], hid_r[:, hid_next])
            hid_next += 1

        ones_sb = singles.tile([1, P], FP16)
        nc.vector.memset(ones_sb[:], 1.0)

        # ---- u broadcast across partitions via PE: u2_rep (128, 8, 256),
        # u1_rep (128, 2, 256), c128 (128, 1) ----
        u1_rep = singles.tile([P, 2, H], FP16)
        u1_ps = ps_pre.tile([P, 2 * H], FP32, tag="pre")
        for kk in range(2):
            nc.tensor.matmul(
                out=u1_ps[:, kk * H : (kk + 1) * H],
                lhsT=ones_sb[:],
                rhs=ucat_sb[:, H : 2 * H],
                start=True,
                stop=True,
            )
        nc.scalar.copy(u1_rep[:].rearrange("p a h -> p (a h)"), u1_ps[:])
        u2_rep = singles.tile([P, 8, H], FP16)
        u2_flat = u2_rep[:].rearrange("p a h -> p (a h)")
        for half in range(2):
            ub_ps = ps_pre.tile([P, 4 * H], FP32, tag="pre")
            for kk in range(4):
                nc.tensor.matmul(
                    out=ub_ps[:, kk * H : (kk + 1) * H],
                    lhsT=ones_sb[:],
                    rhs=ucat_sb[:, 0:H],
                    start=True,
                    stop=True,
                )
            nc.scalar.copy(u2_flat[:, half * 4 * H : (half + 1) * 4 * H], ub_ps[:])
        c_ps = ps_tr.tile([P, 1], FP32, tag="tr")
        nc.tensor.matmul(out=c_ps[:], lhsT=ones_sb[:], rhs=c_sb[:], start=True, stop=True)
        c128 = singles.tile([P, 1], FP32)
        nc.vector.tensor_copy(c128[:], c_ps[:])

        shc_all = singles.tile([P, B_LOC * 2], FP16)

        def hid_dots(b):
            prod2 = scratch_pool.tile([P, 2, H], FP16, tag="prod2")
            nc.vector.tensor_mul(prod2[:], hid_all[:, b], u1_rep[:])
            with nc.allow_low_precision(reason="fp16 dot, tol 2e-2"):
                nc.vector.tensor_reduce(
                    out=shc_all[:, 2 * b : 2 * b + 2],
                    in_=prod2[:],
                    axis=AX.X,
                    op=ALU.add,
                )
            # per-b +c so tanh(b) does not wait on other batches' hid dots
            nc.vector.tensor_scalar_add(
                shc_all[:, 2 * b : 2 * b + 2],
                shc_all[:, 2 * b : 2 * b + 2],
                c128[:, 0:1],
            )

        # ---- main pipeline; finalize (recip+norm+store) for batch
        # b-SKEW is emitted after the dot/PE/ACT stages of batch b so the
        # in-order DVE queue never stalls on ACT ----
        SKEW = 2
        sums_t, xts = {}, {}

        def finalize(b):
            # per-j so each half stores as soon as its sums land; smaller
            # final stores shorten the drain tail
            rsums = stats.tile([P, 2], FP32, tag="rsums")
            o2_t = ot_pool.tile([P, 2, T_LEN], FP16)
            for j in range(2):
                nc.vector.reciprocal(
                    rsums[:, j : j + 1], sums_t[b][:, j : j + 1]
                )
                if b == B_LOC - 1 and j == 1:
                    # very last chain: halve the norm+store so the final
                    # store's bytes start flying sooner
                    for hh in range(2):
                        sl = slice(hh * (T_LEN // 2), (hh + 1) * (T_LEN // 2))
                        nc.vector.tensor_scalar_mul(
                            o2_t[:, j, sl], xts[b][:, j, sl], rsums[:, j : j + 1]
                        )
                        nc.sync.dma_start(out_r[b, :, j, sl], o2_t[:, j, sl])
                else:
                    nc.vector.tensor_scalar_mul(
                        o2_t[:, j, :], xts[b][:, j, :], rsums[:, j : j + 1]
                    )
                    nc.sync.dma_start(out_r[b, :, j], o2_t[:, j, :])

        def enc_dots(b, stats_b, klo, khi):
            nk = khi - klo
            enc_sl = enc_sbs[b][:, klo:khi]
            prod = scratch_pool.tile([P, 8, H], FP16, tag="prod")
            nc.vector.tensor_mul(prod[:, klo:khi], enc_sl, u2_rep[:, klo:khi])
            # fold halves at 2x TT rate, then 1x reduce on half the data
            prodh = scratch_pool.tile([P, 8, H // 2], FP16, tag="prodh")
            nc.vector.tensor_add(
                prodh[:, klo:khi],
                prod[:, klo:khi, 0 : H // 2],
                prod[:, klo:khi, H // 2 : H],
            )
            with nc.allow_low_precision(reason="fp16 dot, tol 2e-2"):
                nc.vector.tensor_reduce(
                    out=stats_b[:, klo:khi],
                    in_=prodh[:, klo:khi],
                    axis=AX.X,
                    op=ALU.add,
                )

        for b in range(B_LOC):
            enc_sb = enc_sbs[b]
            # se dots on DVE, all-fp16 for 2x perf mode; batch 0/1 chunked
            # to chase their split loads
            stats_b = stats.tile([P, 8], FP16, tag="stats")
            if b == 0:
                for lo, hi in ENC0_SPLITS:
                    enc_dots(b, stats_b, lo, hi)
                hid_dots(0)
            elif b == 1:
                for hh in range(2):
                    enc_dots(b, stats_b, 4 * hh, 4 * hh + 4)
            else:
                enc_dots(b, stats_b, 0, 8)

            # se columns -> rows via PE transpose, then one-hot matmuls
            # broadcast them: pre[n_part, t=k*128+t'] = seT[k, t'].
            # b0 runs in two half-groups chasing its quarter-split loads
            # (transpose+broadcast of k0-3 happens while k4-7 still load)
            pre_ps = ps_pre.tile([P, T_LEN], FP32, tag="pre")
            if b == 0:
                for g in range(2):
                    seT_ps = ps_tr.tile([4, P], FP16, tag="tr")
                    nc.tensor.transpose(
                        seT_ps[:], stats_b[:, 4 * g : 4 * g + 4], ident[:]
                    )
                    seT_sb = stats.tile([4, P], FP16, tag="seT")
                    nc.scalar.copy(seT_sb[:], seT_ps[:])
                    for k in range(4 * g, 4 * g + 4):
                        nc.tensor.matmul(
                            out=pre_ps[:, k * P : (k + 1) * P],
                            lhsT=sel4[:, k, :],
                            rhs=seT_sb[:],
                            start=True,
                            stop=True,
                        )
            else:
                seT_ps = ps_tr.tile([8, P], FP16, tag="tr")
                nc.tensor.transpose(seT_ps[:], stats_b[:], ident[:])
                seT_sb = stats.tile([8, P], FP16, tag="seT")
                nc.vector.tensor_copy(seT_sb[:], seT_ps[:])
                for k in range(8):
                    nc.tensor.matmul(
                        out=pre_ps[:, k * P : (k + 1) * P],
                        lhsT=sel[:, k, :],
                        rhs=seT_sb[:],
                        start=True,
                        stop=True,
                    )

            # transcendentals on ACT; exp accumulates row sums
            e_t = et_pool.tile([P, 2, T_LEN], FP16)
            for j in range(2):
                nc.scalar.activation(
                    out=e_t[:, j, :],
                    in_=pre_ps[:],
                    func=AF.Tanh,
                    bias=shc_all[:, 2 * b + j : 2 * b + j + 1],
                    scale=1.0,
                )
            sums = stats.tile([P, 2], FP32, tag="sums")
            x_t = xt_pool.tile([P, 2, T_LEN], FP16)
            for j in range(2):
                nc.scalar.activation(
                    out=x_t[:, j, :],
                    in_=e_t[:, j, :],
                    func=AF.Exp,
                    accum_out=sums[:, j : j + 1],
                )
            sums_t[b] = sums
            xts[b] = x_t

            if b + 1 < B_LOC:
                hid_dots(b + 1)
            if b >= SKEW:
                finalize(b - SKEW)

        for b in range(B_LOC - SKEW, B_LOC):
            finalize(b)

    nc.compile()
    return nc


_CACHE = {}


def get_program():
    if "nc" not in _CACHE:
        _CACHE["nc"] = build_program()
    return _CACHE["nc"]


def make_in_maps(hidden, encoder_outputs, W, b, v):
    hidden = np.asarray(hidden, dtype=np.float32)
    encoder_outputs = np.asarray(encoder_outputs, dtype=np.float32)
    W = np.asarray(W, dtype=np.float32)
    b = np.asarray(b, dtype=np.float32)
    v = np.asarray(v, dtype=np.float32)
    u2 = W[:, H:].T @ v  # (H,)
    u1 = W[:, :H].T @ v  # (H,)
    c = np.float32(b @ v)
    u_cat = np.concatenate([u2, u1]).reshape(1, 2 * H).astype(np.float16)
    c_in = np.full((1, 1), c, dtype=np.float16)
    # one-hot selectors sel[a, k, p] = (a == k); identity for PE transpose
    sel_in = np.zeros((8, 8, P), dtype=np.float16)
    for a in range(8):
        sel_in[a, a, :] = 1.0
    sel_in = sel_in.reshape(8, 8 * P)
    # 4-row variant for batch 0's half-group broadcast: one-hot at k mod 4
    sel4_in = np.zeros((4, 8, P), dtype=np.float16)
    for k in range(8):
        sel4_in[k % 4, k, :] = 1.0
    sel4_in = sel4_in.reshape(4, 8 * P)
    ident_in = np.eye(P, dtype=np.float16)
    in_maps = []
    for i in range(NCORES):
        sl = slice(i * B_LOC, (i + 1) * B_LOC)
        in_maps.append(
            {
                "hidden": np.ascontiguousarray(hidden[:, sl, :]),
                "encoder_outputs": np.ascontiguousarray(encoder_outputs[:, sl, :]),
                "u_cat": u_cat,
                "c_in": c_in,
                "sel_in": sel_in,
                "sel4_in": sel4_in,
                "ident_in": ident_in,
            }
        )
    return in_maps


def kernel(hidden, encoder_outputs, W, b, v, _trace=False, _trace_kwargs=None):
    nc = get_program()
    in_maps = make_in_maps(hidden, encoder_outputs, W, b, v)
    res = run_bass_kernel_spmd(
        nc,
        in_maps,
        core_ids=list(range(NCORES)),
        trace=_trace,
        **(_trace_kwargs or {}),
    )
    out = np.concatenate(
        [res.results[i]["out"].astype(np.float32) for i in range(NCORES)], axis=0
    )
    if _trace:
        return out, res
    return out


# revision 89
# speedup vs baseline: 1.0171x; 1.0171x over previous
"""Trainium2 Bass kernel for Bahdanau-style attention scoring (sparse_attention).

Math (per reference):
    u1 = W[:, :H].T @ v ; u2 = W[:, H:].T @ v ; c = b @ v   (host, tiny)
    sh[b, n] = hidden[n, b, :] @ u1
    se[b, t] = encoder_outputs[t, b, :] @ u2
    out[b, n, t] = softmax_t(tanh(sh[b, n] + se[b, t] + c))

Sharding: data-parallel over batch B=64 across 8 cores (8 batch rows per
core); u1/u2/c replicated. No collectives.

v2 design (from v1 trace: DVE 53.8us / DMA 53us / ACT 52.3us all ~60%
busy, span 84us):
  - fp16 everywhere on chip (DVE 2x perf mode needs all-2B operands);
    fp16 output store halves store traffic: DMA ~53 -> ~42us.
  - enc/hid loaded fp32->fp16 via SWDGE cast-DMA (gpsimd), all up front.
  - DVE: dots at 2x (mul + grouped reduce, fp16 out under
    allow_low_precision), seT copy, recip, normalize j=0.
  - ACT: tanh (bias=sh+c per j) + exp with accum_out row sums.
  - GpSimd: normalize j=1 (tensor_scalar_mul), SWDGE descgen.
  - PE: u broadcast, se transpose, one-hot broadcast matmuls (fp16 1-pass).
"""

import os
import sys

import numpy as np

for _p in ("/opt/trn_rl_repo", "/root/.axon_site/_ro/trn_rl_repo"):
    if os.path.isdir(_p) and _p not in sys.path:
        sys.path.insert(0, _p)

from contextlib import ExitStack

import concourse.bass as bass
import concourse.tile as tile
from concourse import bacc, mybir
from concourse import masks
from concourse.bass_utils import run_bass_kernel_spmd

H = 256
N_LEN = 256
T_LEN = 1024
BATCH = 64
NCORES = 8
B_LOC = BATCH // NCORES  # 8
P = 128
FP32 = mybir.dt.float32
FP16 = mybir.dt.float16
AF = mybir.ActivationFunctionType
ALU = mybir.AluOpType
AX = mybir.AxisListType


def build_program():
    nc = bacc.Bacc(
        "TRN2",
        target_bir_lowering=False,
        debug=False,
        enable_asserts=True,
        num_devices=NCORES,
    )

    hid_ap = nc.dram_tensor("hidden", [N_LEN, B_LOC, H], FP32, kind="ExternalInput").ap()
    enc_ap = nc.dram_tensor(
        "encoder_outputs", [T_LEN, B_LOC, H], FP32, kind="ExternalInput"
    ).ap()
    ucat_ap = nc.dram_tensor("u_cat", [1, 2 * H], FP16, kind="ExternalInput").ap()
    c_ap = nc.dram_tensor("c_in", [1, 1], FP16, kind="ExternalInput").ap()
    sel_ap = nc.dram_tensor("sel_in", [8, 8 * P], FP16, kind="ExternalInput").ap()
    sel4_ap = nc.dram_tensor("sel4_in", [4, 8 * P], FP16, kind="ExternalInput").ap()
    ident_ap = nc.dram_tensor("ident_in", [P, P], FP16, kind="ExternalInput").ap()
    out_ap = nc.dram_tensor(
        "out", [B_LOC, N_LEN, T_LEN], FP16, kind="ExternalOutput"
    ).ap()

    # DRAM views: partition index innermost of the row dims.
    hid_r = hid_ap.rearrange("(j p) b h -> p b j h", p=P)  # (128, 8, 2, 256)
    enc_r = enc_ap.rearrange("(k p) b h -> p b k h", p=P)  # (128, 8, 8, 256)
    out_r = out_ap.rearrange("b (j p) t -> b p j t", p=P)  # (8, 128, 2, 1024)

    with tile.TileContext(nc) as tc, ExitStack() as ctx:
        singles = ctx.enter_context(tc.tile_pool(name="singles", bufs=1))
        ps_pre = ctx.enter_context(tc.tile_pool(name="ps_pre", bufs=3, space="PSUM"))
        ps_tr = ctx.enter_context(tc.tile_pool(name="ps_tr", bufs=2, space="PSUM"))
        enc_pool = ctx.enter_context(tc.tile_pool(name="enc", bufs=8))
        stats = ctx.enter_context(tc.tile_pool(name="stats", bufs=6))
        scratch_pool = ctx.enter_context(tc.tile_pool(name="scratch", bufs=2))
        et_pool = ctx.enter_context(tc.tile_pool(name="et", bufs=2))
        xt_pool = ctx.enter_context(tc.tile_pool(name="xt", bufs=3))
        ot_pool = ctx.enter_context(tc.tile_pool(name="ot", bufs=3))

        # ---- tiny loads first (HWDGE): u_cat row + c scalar ----
        ucat_sb = singles.tile([1, 2 * H], FP16)
        nc.sync.dma_start(ucat_sb[:], ucat_ap)
        c_sb = singles.tile([1, 1], FP16)
        nc.sync.dma_start(c_sb[:], c_ap)
        sel = singles.tile([8, 8, P], FP16)
        nc.sync.dma_start(sel[:].rearrange("a k p -> a (k p)"), sel_ap)
        sel4 = singles.tile([4, 8, P], FP16)
        nc.sync.dma_start(sel4[:].rearrange("a k p -> a (k p)"), sel4_ap)
        ident = singles.tile([P, P], FP16)
        nc.sync.dma_start(ident[:], ident_ap)

        # ---- big input loads, all up front (SWDGE cast fp32->fp16).
        # SWDGE DMAs drain in issue order and each has ~1us descgen plus
        # ~2us completion latency, so batch 0's enc is split in quarters
        # (first dot work can start ~11us in) and batch 1's in halves;
        # hid slices interleave so hid_b lands just before tanh_b needs
        # its bias ----
        hid_all = singles.tile([P, B_LOC, 2, H], FP16)
        enc_sbs = []
        for b in range(B_LOC):
            enc_sb = enc_pool.tile([P, 8, H], FP16)
            enc_sbs.append(enc_sb)
        nc.gpsimd.dma_start(hid_all[:, 0], hid_r[:, 0])
        ENC0_SPLITS = ((0, 2), (2, 4), (4, 6), (6, 8))
        for lo, hi in ENC0_SPLITS:
            nc.gpsimd.dma_start(enc_sbs[0][:, lo:hi], enc_r[:, 0, lo:hi])
        for hh in range(2):
            nc.gpsimd.dma_start(
                enc_sbs[1][:, 4 * hh : 4 * hh + 4], enc_r[:, 1, 4 * hh : 4 * hh + 4]
            )
        hid_next = 1
        for b in range(2, B_LOC):
            for _ in range(2):
                if hid_next < B_LOC:
                    nc.gpsimd.dma_start(hid_all[:, hid_next], hid_r[:, hid_next])
                    hid_next += 1
            nc.gpsimd.dma_start(enc_sbs[b][:], enc_r[:, b])
        while hid_next < B_LOC:
            nc.gpsimd.dma_start(hid_all[:, hid_next], hid_r[:, hid_next])
            hid_next += 1

        ones_sb = singles.tile([1, P], FP16)
        nc.vector.memset(ones_sb[:], 1.0)

        # ---- u broadcast across partitions via PE: u2_rep (128, 8, 256),
        # u1_rep (128, 2, 256), c128 (128, 1) ----
        u1_rep = singles.tile([P, 2, H], FP16)
        u1_ps = ps_pre.tile([P, 2 * H], FP32, tag="pre")
        for kk in range(2):
            nc.tensor.matmul(
                out=u1_ps[:, kk * H : (kk + 1) * H],
                lhsT=ones_sb[:],
                rhs=ucat_sb[:, H : 2 * H],
                start=True,
                stop=True,
            )
        nc.scalar.copy(u1_rep[:].rearrange("p a h -> p (a h)"), u1_ps[:])
        u2_rep = singles.tile([P, 8, H], FP16)
        u2_flat = u2_rep[:].rearrange("p a h -> p (a h)")
        for half in range(2):
            ub_ps = ps_pre.tile([P, 4 * H], FP32, tag="pre")
            for kk in range(4):
                nc.tensor.matmul(
                    out=ub_ps[:, kk * H : (kk + 1) * H],
                    lhsT=ones_sb[:],
                    rhs=ucat_sb[:, 0:H],
                    start=True,
                    stop=True,
                )
            nc.scalar.copy(u2_flat[:, half * 4 * H : (half + 1) * 4 * H], ub_ps[:])
        c_ps = ps_tr.tile([P, 1], FP32, tag="tr")
        nc.tensor.matmul(out=c_ps[:], lhsT=ones_sb[:], rhs=c_sb[:], start=True, stop=True)
        c128 = singles.tile([P, 1], FP32)
        nc.vector.tensor_copy(c128[:], c_ps[:])

        shc_all = singles.tile([P, B_LOC * 2], FP16)

        def hid_dots(b):
            prod2 = scratch_pool.tile([P, 2, H], FP16, tag="prod2")
            nc.vector.tensor_mul(prod2[:], hid_all[:, b], u1_rep[:])
            with nc.allow_low_precision(reason="fp16 dot, tol 2e-2"):
                nc.vector.tensor_reduce(
                    out=shc_all[:, 2 * b : 2 * b + 2],
                    in_=prod2[:],
                    axis=AX.X,
                    op=ALU.add,
                )
            # per-b +c so tanh(b) does not wait on other batches' hid dots
            nc.vector.tensor_scalar_add(
                shc_all[:, 2 * b : 2 * b + 2],
                shc_all[:, 2 * b : 2 * b + 2],
                c128[:, 0:1],
            )

        # ---- main pipeline; finalize (recip+norm+store) for batch
        # b-SKEW is emitted after the dot/PE/ACT stages of batch b so the
        # in-order DVE queue never stalls on ACT ----
        SKEW = 2
        sums_t, xts = {}, {}

        def finalize(b):
            # per-j so each half stores as soon as its sums land; smaller
            # final stores shorten the drain tail
            rsums = stats.tile([P, 2], FP32, tag="rsums")
            o2_t = ot_pool.tile([P, 2, T_LEN], FP16)
            for j in range(2):
                nc.vector.reciprocal(
                    rsums[:, j : j + 1], sums_t[b][:, j : j + 1]
                )
                if b == B_LOC - 1 and j == 1:
                    # very last chain: halve the norm+store so the final
                    # store's bytes start flying sooner
                    for hh in range(2):
                        sl = slice(hh * (T_LEN // 2), (hh + 1) * (T_LEN // 2))
                        nc.vector.tensor_scalar_mul(
                            o2_t[:, j, sl], xts[b][:, j, sl], rsums[:, j : j + 1]
                        )
                        nc.sync.dma_start(out_r[b, :, j, sl], o2_t[:, j, sl])
                else:
                    nc.vector.tensor_scalar_mul(
                        o2_t[:, j, :], xts[b][:, j, :], rsums[:, j : j + 1]
                    )
                    nc.sync.dma_start(out_r[b, :, j], o2_t[:, j, :])

        def enc_dots(b, stats_b, klo, khi):
            nk = khi - klo
            enc_sl = enc_sbs[b][:, klo:khi]
            prod = scratch_pool.tile([P, 8, H], FP16, tag="prod")
            nc.vector.tensor_mul(prod[:, klo:khi], enc_sl, u2_rep[:, klo:khi])
            # fold halves at 2x TT rate, then 1x reduce on half the data
            prodh = scratch_pool.tile([P, 8, H // 2], FP16, tag="prodh")
            nc.vector.tensor_add(
                prodh[:, klo:khi],
                prod[:, klo:khi, 0 : H // 2],
                prod[:, klo:khi, H // 2 : H],
            )
            with nc.allow_low_precision(reason="fp16 dot, tol 2e-2"):
                nc.vector.tensor_reduce(
                    out=stats_b[:, klo:khi],
                    in_=prodh[:, klo:khi],
                    axis=AX.X,
                    op=ALU.add,
                )

        for b in range(B_LOC):
            enc_sb = enc_sbs[b]
            # se dots on DVE, all-fp16 for 2x perf mode; batch 0/1 chunked
            # to chase their split loads
            stats_b = stats.tile([P, 8], FP16, tag="stats")
            if b == 0:
                for lo, hi in ENC0_SPLITS:
                    enc_dots(b, stats_b, lo, hi)
                hid_dots(0)
            elif b == 1:
                for hh in range(2):
                    enc_dots(b, stats_b, 4 * hh, 4 * hh + 4)
            else:
                enc_dots(b, stats_b, 0, 8)

            # se columns -> rows via PE transpose, then one-hot matmuls
            # broadcast them: pre[n_part, t=k*128+t'] = seT[k, t'].
            # b0 runs in two half-groups chasing its quarter-split loads
            # (transpose+broadcast of k0-3 happens while k4-7 still load)
            pre_ps = ps_pre.tile([P, T_LEN], FP32, tag="pre")
            if b == 0:
                for g in range(2):
                    seT_ps = ps_tr.tile([4, P], FP16, tag="tr")
                    nc.tensor.transpose(
                        seT_ps[:], stats_b[:, 4 * g : 4 * g + 4], ident[:]
                    )
                    seT_sb = stats.tile([4, P], FP16, tag="seT")
                    nc.scalar.copy(seT_sb[:], seT_ps[:])
                    for k in range(4 * g, 4 * g + 4):
                        nc.tensor.matmul(
                            out=pre_ps[:, k * P : (k + 1) * P],
                            lhsT=sel4[:, k, :],
                            rhs=seT_sb[:],
                            start=True,
                            stop=True,
                        )
            else:
                seT_ps = ps_tr.tile([8, P], FP16, tag="tr")
                nc.tensor.transpose(seT_ps[:], stats_b[:], ident[:])
                seT_sb = stats.tile([8, P], FP16, tag="seT")
                nc.vector.tensor_copy(seT_sb[:], seT_ps[:])
                for k in range(8):
                    nc.tensor.matmul(
                        out=pre_ps[:, k * P : (k + 1) * P],
                        lhsT=sel[:, k, :],
                        rhs=seT_sb[:],
                        start=True,
                        stop=True,
                    )

            # transcendentals on ACT; exp accumulates row sums
            e_t = et_pool.tile([P, 2, T_LEN], FP16)
            for j in range(2):
                nc.scalar.activation(
                    out=e_t[:, j, :],
                    in_=pre_ps[:],
                    func=AF.Tanh,
                    bias=shc_all[:, 2 * b + j : 2 * b + j + 1],
                    scale=1.0,
                )
            sums = stats.tile([P, 2], FP32, tag="sums")
            x_t = xt_pool.tile([P, 2, T_LEN], FP16)
            for j in range(2):
                nc.scalar.activation(
                    out=x_t[:, j, :],
                    in_=e_t[:, j, :],
                    func=AF.Exp,
                    accum_out=sums[:, j : j + 1],
                )
            sums_t[b] = sums
            xts[b] = x_t

            if b + 1 < B_LOC:
                hid_dots(b + 1)
            if b >= SKEW:
                finalize(b - SKEW)

        for b in range(B_LOC - SKEW, B_LOC):
            finalize(b)

    nc.compile()
    return nc


_CACHE = {}


def get_program():
    if "nc" not in _CACHE:
        _CACHE["nc"] = build_program()
    return _CACHE["nc"]


def make_in_maps(hidden, encoder_outputs, W, b, v):
    hidden = np.asarray(hidden, dtype=np.float32)
    encoder_outputs = np.asarray(encoder_outputs, dtype=np.float32)
    W = np.asarray(W, dtype=np.float32)
    b = np.asarray(b, dtype=np.float32)
    v = np.asarray(v, dtype=np.float32)
    u2 = W[:, H:].T @ v  # (H,)
    u1 = W[:, :H].T @ v  # (H,)
    c = np.float32(b @ v)
    u_cat = np.concatenate([u2, u1]).reshape(1, 2 * H).astype(np.float16)
    c_in = np.full((1, 1), c, dtype=np.float16)
    # one-hot selectors sel[a, k, p] = (a == k); identity for PE transpose
    sel_in = np.zeros((8, 8, P), dtype=np.float16)
    for a in range(8):
        sel_in[a, a, :] = 1.0
    sel_in = sel_in.reshape(8, 8 * P)
    # 4-row variant for batch 0's half-group broadcast: one-hot at k mod 4
    sel4_in = np.zeros((4, 8, P), dtype=np.float16)
    for k in range(8):
        sel4_in[k % 4, k, :] = 1.0
    sel4_in = sel4_in.reshape(4, 8 * P)
    ident_in = np.eye(P, dtype=np.float16)
    in_maps = []
    for i in range(NCORES):
        sl = slice(i * B_LOC, (i + 1) * B_LOC)
        in_maps.append(
            {
                "hidden": np.ascontiguousarray(hidden[:, sl, :]),
                "encoder_outputs": np.ascontiguousarray(encoder_outputs[:, sl, :]),
                "u_cat": u_cat,
                "c_in": c_in,
                "sel_in": sel_in,
                "sel4_in": sel4_in,
                "ident_in": ident_in,
            }
        )
    return in_maps


def kernel(hidden, encoder_outputs, W, b, v, _trace=False, _trace_kwargs=None):
    nc = get_program()
    in_maps = make_in_maps(hidden, encoder_outputs, W, b, v)
    res = run_bass_kernel_spmd(
        nc,
        in_maps,
        core_ids=list(range(NCORES)),
        trace=_trace,
        **(_trace_kwargs or {}),
    )
    out = np.concatenate(
        [res.results[i]["out"].astype(np.float32) for i in range(NCORES)], axis=0
    )
    if _trace:
        return out, res
    return out
